# revision 31
# baseline (speedup 1.0000x reference)
"""GAU attention (gated attention unit) Trainium2 Bass kernel.

Reference computation (B=2, S=2048, D=1024, H=16, DH=64):
    q = (hs @ Wq + bq), k = (hs @ Wk + bk), v = (hs @ Wv + bv)   per-head [B,S,H,DH]
    scores = q k^T / sqrt(DH);  probs = softmax(scores, axis=k)
    gating = sigmoid(gf * mean_d(hs) + gb)          # [B, S] per (batch, query)
    ctx = (probs * gating) @ v;  out = ctx @ Wo + bo

Sharding: 8 cores = 2 batches x 4 head-groups (4 heads each).  Each core
computes out^T partial [D, S] for its (batch, head-group); host sums the 4
partials per batch and adds bo.

v2 dataflow: the kernel is paced by the ACT engine (128 exp instructions of
[128,1024] at ~1us each, one per (qchunk, headpair, ktile)).  All other work
(Q/K/V/O projections, softmax denominators, gating, scaling) is emitted
interleaved into the exp-paced loop so PE/DVE fill the slack instead of
running in serial phases between attention blocks:
  - scores^T per (pair, kt): two row-packed K=64 matmuls -> 2-bank PSUM ->
    one wide exp -> et slot of a [128, 4*1024] bf16 "big" tile (4 kt each).
  - denominators: DVE folds the 4-kt big tiles (3 adds/pair instead of 15),
    then 4 accumulating ones128 matmuls broadcast the k-partition sum.
  - AV: col-packed matmuls, V stationary, accumulated over kt in PSUM; the
    first 4 AVs of a pair are emitted late so the previous pair's ctx scale
    (which shares the 2 ctx PSUM banks) never stalls the in-order PE queue.
  - per-pair tail (denom matmuls, reciprocal, gating+denominator scaling) and
    per-chunk O-projection run as "extras" popped inside the NEXT pair's loop.
  - gating sigmoid is computed as 1/(1+exp(-x)) so only the exp ACT table is
    ever loaded (no sigmoid table, no table thrash); host passes [-gb, -gf/D].
PSUM: scores 2banks x3 bufs + ctx 1bank x2 = 8 banks.  All transient matmul
outputs (K/Q/V-proj, denom, gating broadcast, O-proj) share the scores tag.
"""

import sys

for _p in ("/opt/trn_rl_repo", "/root/.axon_site/_ro/trn_rl_repo"):
    if _p not in sys.path:
        sys.path.append(_p)

from contextlib import ExitStack

import ml_dtypes
import numpy as np

import concourse.bass as bass
import concourse.mybir as mybir
import concourse.tile as tile
from concourse import bacc
from concourse.bass_utils import run_bass_kernel_spmd

BF16 = mybir.dt.bfloat16
F32 = mybir.dt.float32
AF = mybir.ActivationFunctionType
OP = mybir.AluOpType

B, S, D, H = 2, 2048, 1024, 16
DH = 64
HPC = 4  # heads per core
GD = HPC * DH  # 256 (head-group width)
NCORES = 8
NDT = D // 128  # 8 contraction tiles over D
GQ = 512  # q-chunk width
NQC = S // GQ  # 4 q chunks
NKT = S // 128  # 16 k tiles
BKT = 4  # k tiles per "big" et tile
NBIG = NKT // BKT  # 4


def _build(ctx: ExitStack, tc: "tile.TileContext", io: dict):
    nc = tc.nc
    hsT, wq, wk, wv, wo = io["hsT"], io["wq"], io["wk"], io["wv"], io["wo"]
    bq, bk, bv, gg, outT = io["bq"], io["bk"], io["bv"], io["gg"], io["outT"]

    consts = ctx.enter_context(tc.tile_pool(name="consts", bufs=1))
    sb = ctx.enter_context(tc.tile_pool(name="sb", bufs=1))
    etp = ctx.enter_context(tc.tile_pool(name="etp", bufs=3))
    wrk = ctx.enter_context(tc.tile_pool(name="wrk", bufs=2))
    # PSUM: smm [128,1024] = 2 banks x 3 bufs + ctx [128,512] = 1 bank x 2
    ps_s = ctx.enter_context(tc.tile_pool(name="ps_s", bufs=3, space="PSUM"))
    ps_c = ctx.enter_context(tc.tile_pool(name="ps_c", bufs=2, space="PSUM"))

    # ---- constants (memsets only; const DMAs go after the big loads —
    # the sync HWDGE ring is FIFO, so small DMAs first would delay them) ----
    ones128 = consts.tile([128, 128], BF16, tag="ones128", name="ones128")
    nc.vector.memset(ones128[:], 1.0)
    ones1b = consts.tile([1, 128], BF16, tag="ones1b", name="ones1b")
    nc.vector.memset(ones1b[:], 1.0)
    zbias = consts.tile([128, 1], F32, tag="zbias", name="zbias")
    nc.vector.memset(zbias[:], 0.0)

    # ---- load weights first (the head K/Q projections need them before the
    # bulk of hs^T), then hs^T in two column-halves.  All tensors arrive
    # host-tiled [128, d, cols] so each matrix is ONE large DMA with >=4KB
    # per-partition contiguous runs (DMA is descriptor-dominated below 1MB).
    wk_all = consts.tile([128, NDT, GD], BF16, tag="wk", name="wk_all")
    nc.sync.dma_start(wk_all[:], wk[:])
    wq_all = consts.tile([128, NDT, GD], BF16, tag="wq", name="wq_all")
    nc.sync.dma_start(wq_all[:], wq[:])
    wv_all = consts.tile([128, NDT, GD], BF16, tag="wv", name="wv_all")
    nc.sync.dma_start(wv_all[:], wv[:])
    wk_sb = [wk_all[:, d, :] for d in range(NDT)]
    wq_sb = [wq_all[:, d, :] for d in range(NDT)]
    wv_sb = [wv_all[:, d, :] for d in range(NDT)]
    # small consts used early in pair 0 go before hsT (all tiny)
    bv_bc = consts.tile([128, GD], F32, tag="bvbc", name="bvbc")
    nc.sync.dma_start(bv_bc[:], bv[:, :])
    gg_sb = consts.tile([1, 2], F32, tag="gg", name="gg")
    nc.sync.dma_start(gg_sb[:], gg[None, :])
    bq_sb = consts.tile([128, 2], F32, tag="bq", name="bq")
    nc.sync.dma_start(bq_sb[:], bq[:])
    bk_sb = consts.tile([128, 2], F32, tag="bk", name="bk")
    nc.sync.dma_start(bk_sb[:], bk[:])
    # hsT in 4 column-quarter DMAs (1MB each) so the head projections can
    # start on quarter 0 while the rest streams in
    hsT_all = sb.tile([128, NDT, S], BF16, tag="hsT", name="hsT_all")
    for h in range(4):
        nc.sync.dma_start(hsT_all[:, :, h * 512 : (h + 1) * 512], hsT[h])
    hsT_sb = [hsT_all[:, d, :] for d in range(NDT)]
    wo_all = consts.tile([128, 2, D], BF16, tag="wo", name="wo_all")
    nc.sync.dma_start(wo_all[:], wo[:])
    wo_sb = [wo_all[:, p, :] for p in range(2)]

    # ---- PE warmup: dummy matmuls (no DMA deps) so HAM reaches 8/8 before
    # the real head matmuls issue; they run during the initial DMA wait ----
    warm = ps_s.tile([128, 128], F32, tag="smm", padded_shape=[128, 1024], name="warm")
    for i in range(28):
        nc.tensor.matmul(warm[:], lhsT=ones128[:], rhs=ones128[:], start=True, stop=True)

    qT_sb = [sb.tile([128, S], BF16, tag=f"qT{m}", name=f"qT{m}") for m in range(2)]
    kT_sb = [sb.tile([128, S], BF16, tag=f"kT{m}", name=f"kT{m}") for m in range(2)]
    v_sb = [sb.tile([128, GD], BF16, tag=f"v{st}", name=f"v{st}") for st in range(NKT)]
    # gating broadcast [128, 2*GQ] per 2-qc group; per-parity tiles
    gb_sb = [sb.tile([128, 2 * GQ], F32, tag=f"gb{h}", name=f"gb{h}") for h in range(2)]
    # scaled ctx^T bf16, alive into the following qc (O-projection)
    ctx_sc = [
        [sb.tile([128, GQ], BF16, tag=f"ctxs{p}_{par}", name=f"ctxs{p}_{par}") for p in range(2)]
        for par in range(2)
    ]

    def qk_proj(w_sb, dst, m, cols, bias_sb):
        """Project [128,1024] of Q^T or K^T (m selects the 128-row pair tile).
        A matmul output must fit one PSUM bank, so each 512-col half is its
        own accumulation group; the bias-add cast reads both banks at once."""
        p = ps_s.tile([128, 1024], F32, tag="smm", name="pp")
        for h in range(2):
            hs_ = slice(cols.start + h * 512, cols.start + (h + 1) * 512)
            for d in range(NDT):
                nc.tensor.matmul(
                    p[:, h * 512 : (h + 1) * 512],
                    lhsT=w_sb[d][:, m * 128 : (m + 1) * 128], rhs=hsT_sb[d][:, hs_],
                    start=(d == 0), stop=(d == NDT - 1),
                )
        nc.vector.tensor_scalar_add(dst[:, cols], p[:], bias_sb[:, m : m + 1])

    def v_proj(st, half):
        """Project the 128 V columns a single head-pair needs (half=pr), so
        each of the two qc0 pairs computes just its own half JIT."""
        vp = ps_s.tile([128, 128], F32, tag="smm", padded_shape=[128, 1024], name="vp")
        ss = slice(st * 128, (st + 1) * 128)
        hc = slice(half * 128, (half + 1) * 128)
        for d in range(NDT):
            nc.tensor.matmul(
                vp[:], lhsT=hsT_sb[d][:, ss], rhs=wv_sb[d][:, hc],
                start=(d == 0), stop=(d == NDT - 1),
            )
        nc.vector.tensor_tensor(v_sb[st][:, hc], vp[:], bv_bc[:, hc], op=OP.add)

    # ---- main loop: 8 pairs x 16 exp-paced slots, with interleaved extras ----
    # state carried between pairs for the deferred tail
    prev = None  # (qc, pr, ks2, ctxA, ctxB)

    def make_tail_extras(qc, pr, ks2, ctxA, ctxB, done=None):
        """Denominator matmuls + reciprocal + gating/denominator ctx scaling
        for pair (qc, pr), emitted as extras inside the following pair."""
        ex = []
        dn = {}

        def khalf():
            # collapse ks2's two slot-pairs with one wide DVE add; halves the
            # number of denominator matmul streams the PE must run
            dn["kh"] = wrk.tile([128, 2048], BF16, tag="kh", name="kh")
            nc.vector.tensor_tensor(dn["kh"][:], ks2[:, 0:2048], ks2[:, 2048:4096], op=OP.add)
        ex.append((1, 2, khalf))

        def dmm(j):
            def f():
                if j == 0:
                    dn["t"] = ps_s.tile([128, 1024], F32, tag="smm", name="dn")
                for h in range(2):
                    nc.tensor.matmul(
                        dn["t"][:, h * 512 : (h + 1) * 512], lhsT=ones128[:],
                        rhs=dn["kh"][:, j * 1024 + h * 512 : j * 1024 + (h + 1) * 512],
                        start=(j == 0), stop=(j == 1),
                    )
            return f

        for j in range(2):
            ex.append((2, 2, dmm(j)))

        def recip():
            dn["r"] = wrk.tile([128, 1024], F32, tag="rab", name="rab")
            nc.vector.reciprocal_approx_fast(dn["r"][:], dn["t"][:])
        ex.append((1, 2, recip))

        def scale1():
            dn["tmp"] = wrk.tile([128, GQ], F32, tag="tmp", name="tmp")
            nc.vector.tensor_tensor(
                dn["tmp"][0:64, :], ctxA[0:64, :], dn["r"][0:64, 0:GQ], op=OP.mult
            )

        def scale2():
            nc.vector.tensor_tensor(
                dn["tmp"][64:128, :], ctxB[64:128, :], dn["r"][64:128, GQ : 2 * GQ],
                op=OP.mult,
            )

        def scale3():
            gcols = slice((qc % 2) * GQ, (qc % 2) * GQ + GQ)
            nc.vector.tensor_tensor(
                ctx_sc[qc % 2][pr][:], dn["tmp"][:], gb_sb[(qc // 2) % 2][:, gcols],
                op=OP.mult,
            )
            if done is not None:
                done[0] = True
        ex += [(1, 2, scale1), (1, 2, scale2), (1, 2, scale3)]
        return ex

    def make_oproj_extras(qc, mts):
        """O-projection of chunk qc (reads ctx_sc[qc%2][*]) for column tiles
        `mts`, two extras each (matmul pair, then cast+store)."""
        ex = []
        cs = slice(qc * GQ, (qc + 1) * GQ)
        st = {}
        for mt in mts:
            def fmm(mt=mt):
                ms = slice(mt * 128, (mt + 1) * 128)
                o_ps = ps_s.tile(
                    [128, GQ], F32, tag="smm", padded_shape=[128, 1024], name="ops"
                )
                for p in range(2):
                    nc.tensor.matmul(
                        o_ps[:], lhsT=wo_sb[p][:, ms], rhs=ctx_sc[qc % 2][p][:],
                        start=(p == 0), stop=(p == 1),
                    )
                st["ps"] = o_ps

            def fout(mt=mt):
                # pair up column tiles into one [128, 1024] store (bigger
                # per-partition DMA runs -> fewer, fatter descriptors)
                if mt % 2 == 0:
                    st["ost"] = wrk.tile([128, 2 * GQ], F32, tag="ost", bufs=4, name="ost")
                nc.vector.tensor_copy(
                    st["ost"][:, (mt % 2) * GQ : (mt % 2 + 1) * GQ], st["ps"][:]
                )
                if mt % 2 == 1:
                    nc.sync.dma_start(outT[qc, mt // 2, :, :], st["ost"][:])
            ex += [(2, 0, fmm), (1, 0, fout)]
        return ex

    def make_proj_extras(w_sb, bias_sb, dst, m, half, split_cast=False):
        """Q^T/K^T projection m-tile for qc pair `half`, as extras (2 matmul
        half-groups + bias cast; split_cast casts each 512-col half as soon as
        its matmuls finish, so consumers do not wait on the later half's DMA)."""
        cols = slice(half * 1024, (half + 1) * 1024)
        st = {}

        def fh(h):
            def f():
                if h == 0:
                    st["p"] = ps_s.tile([128, 1024], F32, tag="smm", name="pp")
                hs_ = slice(cols.start + h * 512, cols.start + (h + 1) * 512)
                for d in range(NDT):
                    nc.tensor.matmul(
                        st["p"][:, h * 512 : (h + 1) * 512],
                        lhsT=w_sb[d][:, m * 128 : (m + 1) * 128],
                        rhs=hsT_sb[d][:, hs_],
                        start=(d == 0), stop=(d == NDT - 1),
                    )
            return f

        def fcast(h0, h1):
            def f():
                cc = slice(cols.start + h0 * 512, cols.start + h1 * 512)
                nc.vector.tensor_scalar_add(
                    dst[:, cc], st["p"][:, h0 * 512 : h1 * 512], bias_sb[:, m : m + 1]
                )
            return f

        if split_cast:
            return [(4, 0, fh(0)), (1, 0, fcast(0, 1)), (4, 0, fh(1)), (1, 0, fcast(1, 2))]
        return [(4, 0, fh(0)), (4, 0, fh(1)), (1, 0, fcast(0, 2))]

    def make_projq_extras(w_sb, bias_sb, dst, m, q):
        """Single 512-col projection group (quarter q) as [matmuls, cast]."""
        cc = slice(q * 512, (q + 1) * 512)
        st = {}

        def fmm():
            st["p"] = ps_s.tile([128, 512], F32, tag="smm", padded_shape=[128, 1024], name="pq")
            for d in range(NDT):
                nc.tensor.matmul(
                    st["p"][:], lhsT=w_sb[d][:, m * 128 : (m + 1) * 128],
                    rhs=hsT_sb[d][:, cc],
                    start=(d == 0), stop=(d == NDT - 1),
                )

        def fcast():
            nc.vector.tensor_scalar_add(dst[:, cc], st["p"][:], bias_sb[:, m : m + 1])
        return [(4, 0, fmm), (1, 0, fcast)]

    def make_gating_extras(g2):
        """Gating for qc pair g2, split into fine-grained extras."""
        cols = slice(g2 * 1024, (g2 + 1) * 1024)
        st = {}

        def fmean(h):
            def f():
                if h == 0:
                    st["gp"] = ps_s.tile(
                        [1, 1024], F32, tag="smm", padded_shape=[128, 1024], name="gp"
                    )
                hs_ = slice(cols.start + h * 512, cols.start + (h + 1) * 512)
                for d in range(NDT):
                    nc.tensor.matmul(
                        st["gp"][:, h * 512 : (h + 1) * 512], lhsT=ones128[:, 0:1],
                        rhs=hsT_sb[d][:, hs_],
                        start=(d == 0), stop=(d == NDT - 1),
                    )
            return f

        def fexp():
            st["e"] = wrk.tile([1, 1024], F32, tag="ge", name="ge")
            nc.scalar.activation(
                st["e"][:], st["gp"][:], AF.Exp, bias=gg_sb[:, 0:1], scale=gg_sb[:, 1:2]
            )

        def fsig():
            gt = wrk.tile([1, 1024], F32, tag="gt", name="gt")
            nc.vector.tensor_scalar_add(gt[:], st["e"][:], 1.0)
            st["g"] = wrk.tile([1, 1024], F32, tag="gr", name="gr")
            nc.vector.reciprocal_approx_fast(st["g"][:], gt[:])

        def fsplit():
            st["gh"] = wrk.tile([1, 1024], BF16, tag="gh", name="gh")
            nc.vector.tensor_copy(st["gh"][:], st["g"][:])
            gd = wrk.tile([1, 1024], F32, tag="gd", name="gd")
            nc.vector.tensor_tensor(gd[:], st["g"][:], st["gh"][:], op=OP.subtract)
            st["gl"] = wrk.tile([1, 1024], BF16, tag="gl", name="gl")
            nc.vector.tensor_copy(st["gl"][:], gd[:])

        def fbc(h):
            def f():
                if h == 0:
                    st["gbp"] = ps_s.tile([128, 1024], F32, tag="smm", name="gbp")
                hw = slice(h * 512, (h + 1) * 512)
                nc.tensor.matmul(
                    st["gbp"][:, hw], lhsT=ones1b[:], rhs=st["gh"][:, hw],
                    start=True, stop=False,
                )
                nc.tensor.matmul(
                    st["gbp"][:, hw], lhsT=ones1b[:], rhs=st["gl"][:, hw],
                    start=False, stop=True,
                )
            return f

        def fcopy():
            nc.vector.tensor_copy(gb_sb[g2 % 2][:], st["gbp"][:])
        return [(4, 0, fmean(0)), (4, 0, fmean(1)), (1, 0, fexp), (1, 0, fsig),
                (1, 0, fsplit), (1, 0, fbc(0)), (1, 0, fbc(1)), (1, 0, fcopy)]

    # ---- head phase: minimum work before exp #0 can issue ----
    # K^T and Q^T first quarters only (k tiles 0..3, q chunk 0); everything
    # else runs as extras / JIT inside the qc0 pairs.
    for m in range(2):
        for _, _, f in make_projq_extras(wk_sb, bk_sb, kT_sb[m], m, 0):
            f()
    for m in range(2):
        for _, _, f in make_projq_extras(wq_sb, bq_sb, qT_sb[m], m, 0):
            f()

    extras = []  # persistent work queue, popped a few items per slot
    # pair 0: the remaining K/Q quarter-1 groups (k tiles 4..7 / q chunk 1),
    # then K m0's second half (kT_sb[m] is read by pr=m only) and gating(0)
    for m in range(2):
        extras += make_projq_extras(wk_sb, bk_sb, kT_sb[m], m, 1)
    for m in range(2):
        extras += make_projq_extras(wq_sb, bq_sb, qT_sb[m], m, 1)
    extras += make_proj_extras(wk_sb, bk_sb, kT_sb[0], 0, 1, split_cast=True)
    extras += make_gating_extras(0)
    for qc in range(NQC):
        cs = slice(qc * GQ, (qc + 1) * GQ)
        for pr in range(2):
            tail_done = [prev is None]
            if qc == 0 and pr == 1:
                # pr1's K m1 second half must land before its k tile 8
                extras += make_proj_extras(wk_sb, bk_sb, kT_sb[1], 1, 1, split_cast=True)
            if prev is not None:
                extras += make_tail_extras(*prev, done=tail_done)
            if pr == 0 and qc > 0:
                extras += make_oproj_extras(qc - 1, range(0, 4))
            if pr == 1 and qc > 0:
                extras += make_oproj_extras(qc - 1, range(4, 8))
            if qc == 1 and pr == 0:
                extras += make_proj_extras(wq_sb, bq_sb, qT_sb[0], 0, 1)
            if qc == 1 and pr == 1:
                extras += make_proj_extras(wq_sb, bq_sb, qT_sb[1], 1, 1)
            if qc == 2 and pr == 0:
                extras += make_gating_extras(1)
            jit_v = qc == 0

            ctxA = ps_c.tile([128, GQ], F32, tag="ctx", name="ctxA")
            ctxB = ps_c.tile([128, GQ], F32, tag="ctx", name="ctxB")
            ks2 = sb.tile([128, BKT * 1024], BF16, tag=f"ks2_{(qc * 2 + pr) % 2}", name="ks2")
            bigs = [None] * NBIG
            pend_av = []  # AV matmuls deferred while the prev tail drains ctx banks
            first_pair = prev is None

            def av(kt):
                big = bigs[kt // BKT]
                base = (kt % BKT) * 1024
                nc.tensor.matmul(
                    ctxA[0:64, :], lhsT=v_sb[kt][:, pr * 128 : pr * 128 + 64],
                    rhs=big[:, base : base + GQ], tile_position=(0, 0),
                    start=(kt == 0), stop=(kt == NKT - 1),
                )
                nc.tensor.matmul(
                    ctxB[64:128, :], lhsT=v_sb[kt][:, pr * 128 + 64 : pr * 128 + 128],
                    rhs=big[:, base + GQ : base + 2 * GQ], tile_position=(0, 64),
                    start=(kt == 0), stop=(kt == NKT - 1),
                )

            for kt in range(NKT):
                # scores pair -> 2-bank PSUM
                ks_ = slice(kt * 128, (kt + 1) * 128)
                sp = ps_s.tile([128, 2 * GQ], F32, tag="smm", name="smm")
                nc.tensor.matmul(
                    sp[:, 0:GQ], lhsT=kT_sb[pr][0:64, ks_], rhs=qT_sb[pr][0:64, cs],
                    tile_position=(0, 0), start=True, stop=True,
                )
                nc.tensor.matmul(
                    sp[:, GQ : 2 * GQ], lhsT=kT_sb[pr][64:128, ks_], rhs=qT_sb[pr][64:128, cs],
                    tile_position=(64, 0), start=True, stop=True,
                )
                # wide exp into the et big-tile slot
                if kt % BKT == 0:
                    bigs[kt // BKT] = etp.tile(
                        [128, BKT * 1024], BF16, tag="big", name="big"
                    )
                eslot = bigs[kt // BKT][:, (kt % BKT) * 1024 : (kt % BKT + 1) * 1024]
                nc.scalar.activation(eslot, sp[:], AF.Exp, bias=zbias[:, 0:1], scale=0.125)
                # extras fill the PE/DVE slack inside the exp-paced slot.
                # min_slot keeps the previous pair's denominator matmuls
                # (which depend on its last fold) from being emitted ahead of
                # scores(1..2), which would stall the in-order PE queue.
                budget = 3 if jit_v else 4
                while extras and budget > 0 and extras[0][1] <= kt:
                    w, _, f = extras.pop(0)
                    budget -= w
                    f()
                if jit_v:
                    v_proj(kt, pr)
                # AV of the previous kt (deferred while prev ctx banks drain)
                if kt > 0:
                    pend_av.append(kt - 1)
                    if tail_done[0]:
                        while pend_av:
                            av(pend_av.pop(0))
                # fold completed big tiles into ks2, half-width at a time; the
                # last big folds its first half early (after exp 13) so only
                # one [128,2048] add depends on the final exp of the pair.
                HW = BKT * 1024 // 2
                if kt == 2 * BKT - 1:
                    for h in range(2):
                        nc.vector.tensor_tensor(
                            ks2[:, h * HW : (h + 1) * HW], bigs[0][:, h * HW : (h + 1) * HW],
                            bigs[1][:, h * HW : (h + 1) * HW], op=OP.add,
                        )
                elif kt == 3 * BKT - 1:
                    for h in range(2):
                        nc.vector.tensor_tensor(
                            ks2[:, h * HW : (h + 1) * HW], ks2[:, h * HW : (h + 1) * HW],
                            bigs[2][:, h * HW : (h + 1) * HW], op=OP.add,
                        )
                elif kt == NKT - 3:
                    nc.vector.tensor_tensor(
                        ks2[:, 0:HW], ks2[:, 0:HW], bigs[NBIG - 1][:, 0:HW], op=OP.add
                    )
                elif kt == NKT - 1:
                    nc.vector.tensor_tensor(
                        ks2[:, HW:], ks2[:, HW:], bigs[NBIG - 1][:, HW:], op=OP.add
                    )
            while pend_av:
                av(pend_av.pop(0))
            av(NKT - 1)
            prev = (qc, pr, ks2, ctxA, ctxB)

    # ---- drain: leftover extras, last pair tail, last O-projection ----
    for _, _, f in extras + make_tail_extras(*prev) + make_oproj_extras(NQC - 1, range(8)):
        f()


def build_gau_nc(debug: bool = False):
    nc = bacc.Bacc("TRN2", target_bir_lowering=False, debug=debug, num_devices=NCORES)
    io = {
        "hsT": nc.dram_tensor("hsT", [4, 128, NDT, S // 4], BF16, kind="ExternalInput").ap(),
        "wq": nc.dram_tensor("wq", [128, NDT, GD], BF16, kind="ExternalInput").ap(),
        "wk": nc.dram_tensor("wk", [128, NDT, GD], BF16, kind="ExternalInput").ap(),
        "wv": nc.dram_tensor("wv", [128, NDT, GD], BF16, kind="ExternalInput").ap(),
        "wo": nc.dram_tensor("wo", [128, 2, D], BF16, kind="ExternalInput").ap(),
        "bq": nc.dram_tensor("bq", [128, 2], F32, kind="ExternalInput").ap(),
        "bk": nc.dram_tensor("bk", [128, 2], F32, kind="ExternalInput").ap(),
        "bv": nc.dram_tensor("bv", [128, GD], F32, kind="ExternalInput").ap(),
        "gg": nc.dram_tensor("gg", [2], F32, kind="ExternalInput").ap(),
        "outT": nc.dram_tensor("outT", [NQC, D // 256, 128, 2 * GQ], F32, kind="ExternalOutput").ap(),
    }
    with tile.TileContext(nc) as tc:
        with ExitStack() as ctx:
            _build(ctx, tc, io)
    nc.compile()
    return nc


def make_in_maps(hidden_states, Wq, bq, Wk, bk, Wv, bv, Wo, gating_factor, gating_bias):
    """Shard full inputs into 8 per-core input maps (host-side prep)."""
    bf = ml_dtypes.bfloat16
    f32 = np.float32
    hs = np.asarray(hidden_states, f32)
    Wq, Wk, Wv, Wo = (np.asarray(a, f32) for a in (Wq, Wk, Wv, Wo))
    bq, bk, bv = (np.asarray(a, f32) for a in (bq, bk, bv))
    gf = np.float32(np.asarray(gating_factor, f32)[0])
    gb = np.float32(np.asarray(gating_bias, f32)[0])

    # hsT as [2(col half), 128(part), NDT, 1024]: one contiguous block per
    # DMA with 2KB-per-partition runs
    hsT_b = [
        np.ascontiguousarray(
            hs[b].T.reshape(NDT, 128, 4, S // 4).transpose(2, 1, 0, 3)
        ).astype(bf)
        for b in range(B)
    ]

    def wtile(W):  # [D, cols] -> [128, NDT, cols]
        return np.ascontiguousarray(
            W.reshape(NDT, 128, W.shape[1]).transpose(1, 0, 2)
        ).astype(bf)

    in_maps = []
    for c in range(NCORES):
        b, g = divmod(c, NCORES // B)
        cols = slice(g * GD, (g + 1) * GD)
        in_maps.append(
            {
                "hsT": hsT_b[b],
                "wq": wtile(Wq[:, cols]),
                "wk": wtile(Wk[:, cols]),
                "wv": wtile(Wv[:, cols]),
                "wo": np.ascontiguousarray(
                    Wo[cols, :].reshape(2, 128, D).transpose(1, 0, 2)
                ).astype(bf),
                "bq": np.ascontiguousarray(bq[cols].reshape(2, 128).T),
                "bk": np.ascontiguousarray(bk[cols].reshape(2, 128).T),
                "bv": np.ascontiguousarray(np.broadcast_to(bv[cols], (128, GD))),
                "gg": np.array([-gb, -gf / D], f32),
            }
        )
    return in_maps


_NC_CACHE: dict = {}


def _get_nc():
    if "nc" not in _NC_CACHE:
        _NC_CACHE["nc"] = build_gau_nc()
    return _NC_CACHE["nc"]


def run_gau(in_maps, **kwargs):
    nc = _get_nc()
    return run_bass_kernel_spmd(nc, in_maps, core_ids=list(range(NCORES)), **kwargs)


def assemble_output(results, bo):
    """Sum per-batch head-group partials, transpose back, add bo."""
    bo = np.asarray(bo, np.float32)
    gpb = NCORES // B
    out = np.empty((B, S, D), np.float32)
    for b in range(B):
        acc = results[gpb * b]["outT"].astype(np.float32)
        for g in range(1, gpb):
            acc = acc + results[gpb * b + g]["outT"].astype(np.float32)
        # acc is [NQC, D//256, 128, 2*GQ] -> [D, S]
        acc = acc.reshape(NQC, D // 256, 128, 2, GQ).transpose(1, 3, 2, 0, 4).reshape(D, S)
        out[b] = acc.T + bo[None, :]
    return out


def kernel(hidden_states, Wq, bq, Wk, bk, Wv, bv, Wo, bo, gating_factor, gating_bias):
    in_maps = make_in_maps(
        hidden_states, Wq, bq, Wk, bk, Wv, bv, Wo, gating_factor, gating_bias
    )
    res = run_gau(in_maps)
    return assemble_output(res.results, bo)


# revision 32
# speedup vs baseline: 1.0216x; 1.0216x over previous
"""GAU attention (gated attention unit) Trainium2 Bass kernel.

Reference computation (B=2, S=2048, D=1024, H=16, DH=64):
    q = (hs @ Wq + bq), k = (hs @ Wk + bk), v = (hs @ Wv + bv)   per-head [B,S,H,DH]
    scores = q k^T / sqrt(DH);  probs = softmax(scores, axis=k)
    gating = sigmoid(gf * mean_d(hs) + gb)          # [B, S] per (batch, query)
    ctx = (probs * gating) @ v;  out = ctx @ Wo + bo

Sharding: 8 cores = 2 batches x 4 head-groups (4 heads each).  Each core
computes out^T partial [D, S] for its (batch, head-group); host sums the 4
partials per batch and adds bo.

v2 dataflow: the kernel is paced by the ACT engine (128 exp instructions of
[128,1024] at ~1us each, one per (qchunk, headpair, ktile)).  All other work
(Q/K/V/O projections, softmax denominators, gating, scaling) is emitted
interleaved into the exp-paced loop so PE/DVE fill the slack instead of
running in serial phases between attention blocks:
  - scores^T per (pair, kt): two row-packed K=64 matmuls -> 2-bank PSUM ->
    one wide exp -> et slot of a [128, 4*1024] bf16 "big" tile (4 kt each).
  - denominators: DVE folds the 4-kt big tiles (3 adds/pair instead of 15),
    then 4 accumulating ones128 matmuls broadcast the k-partition sum.
  - AV: col-packed matmuls, V stationary, accumulated over kt in PSUM; the
    first 4 AVs of a pair are emitted late so the previous pair's ctx scale
    (which shares the 2 ctx PSUM banks) never stalls the in-order PE queue.
  - per-pair tail (denom matmuls, reciprocal, gating+denominator scaling) and
    per-chunk O-projection run as "extras" popped inside the NEXT pair's loop.
  - gating sigmoid is computed as 1/(1+exp(-x)) so only the exp ACT table is
    ever loaded (no sigmoid table, no table thrash); host passes [-gb, -gf/D].
PSUM: scores 2banks x3 bufs + ctx 1bank x2 = 8 banks.  All transient matmul
outputs (K/Q/V-proj, denom, gating broadcast, O-proj) share the scores tag.
"""

import sys

for _p in ("/opt/trn_rl_repo", "/root/.axon_site/_ro/trn_rl_repo"):
    if _p not in sys.path:
        sys.path.append(_p)

from contextlib import ExitStack

import ml_dtypes
import numpy as np

import concourse.bass as bass
import concourse.mybir as mybir
import concourse.tile as tile
from concourse import bacc
from concourse.bass_utils import run_bass_kernel_spmd

BF16 = mybir.dt.bfloat16
F32 = mybir.dt.float32
AF = mybir.ActivationFunctionType
OP = mybir.AluOpType

B, S, D, H = 2, 2048, 1024, 16
DH = 64
HPC = 4  # heads per core
GD = HPC * DH  # 256 (head-group width)
NCORES = 8
NDT = D // 128  # 8 contraction tiles over D
GQ = 512  # q-chunk width
NQC = S // GQ  # 4 q chunks
NKT = S // 128  # 16 k tiles
BKT = 4  # k tiles per "big" et tile
NBIG = NKT // BKT  # 4


def _build(ctx: ExitStack, tc: "tile.TileContext", io: dict):
    nc = tc.nc
    hsT, wq, wk, wv, wo = io["hsT"], io["wq"], io["wk"], io["wv"], io["wo"]
    bq, bk, bv, gg, outT = io["bq"], io["bk"], io["bv"], io["gg"], io["outT"]

    consts = ctx.enter_context(tc.tile_pool(name="consts", bufs=1))
    sb = ctx.enter_context(tc.tile_pool(name="sb", bufs=1))
    etp = ctx.enter_context(tc.tile_pool(name="etp", bufs=3))
    wrk = ctx.enter_context(tc.tile_pool(name="wrk", bufs=2))
    # PSUM: smm [128,1024] = 2 banks x 3 bufs + ctx [128,512] = 1 bank x 2
    ps_s = ctx.enter_context(tc.tile_pool(name="ps_s", bufs=3, space="PSUM"))
    ps_c = ctx.enter_context(tc.tile_pool(name="ps_c", bufs=2, space="PSUM"))

    # ---- constants (memsets only; const DMAs go after the big loads —
    # the sync HWDGE ring is FIFO, so small DMAs first would delay them) ----
    ones128 = consts.tile([128, 128], BF16, tag="ones128", name="ones128")
    nc.vector.memset(ones128[:], 1.0)
    ones1b = consts.tile([1, 128], BF16, tag="ones1b", name="ones1b")
    nc.vector.memset(ones1b[:], 1.0)
    zbias = consts.tile([128, 1], F32, tag="zbias", name="zbias")
    nc.vector.memset(zbias[:], 0.0)

    # ---- load weights first (the head K/Q projections need them before the
    # bulk of hs^T), then hs^T in two column-halves.  All tensors arrive
    # host-tiled [128, d, cols] so each matrix is ONE large DMA with >=4KB
    # per-partition contiguous runs (DMA is descriptor-dominated below 1MB).
    wk_all = consts.tile([128, NDT, GD], BF16, tag="wk", name="wk_all")
    nc.sync.dma_start(wk_all[:], wk[:])
    wq_all = consts.tile([128, NDT, GD], BF16, tag="wq", name="wq_all")
    nc.sync.dma_start(wq_all[:], wq[:])
    wv_all = consts.tile([128, NDT, GD], BF16, tag="wv", name="wv_all")
    nc.sync.dma_start(wv_all[:], wv[:])
    wk_sb = [wk_all[:, d, :] for d in range(NDT)]
    wq_sb = [wq_all[:, d, :] for d in range(NDT)]
    wv_sb = [wv_all[:, d, :] for d in range(NDT)]
    # small consts used early in pair 0 go before hsT (all tiny)
    bv_bc = consts.tile([128, GD], F32, tag="bvbc", name="bvbc")
    nc.sync.dma_start(bv_bc[:], bv[:, :])
    gg_sb = consts.tile([1, 2], F32, tag="gg", name="gg")
    nc.sync.dma_start(gg_sb[:], gg[None, :])
    bq_sb = consts.tile([128, 2], F32, tag="bq", name="bq")
    nc.sync.dma_start(bq_sb[:], bq[:])
    bk_sb = consts.tile([128, 2], F32, tag="bk", name="bk")
    nc.sync.dma_start(bk_sb[:], bk[:])
    # hsT in 4 column-quarter DMAs (1MB each) so the head projections can
    # start on quarter 0 while the rest streams in
    hsT_all = sb.tile([128, NDT, S], BF16, tag="hsT", name="hsT_all")
    for h in range(4):
        nc.sync.dma_start(hsT_all[:, :, h * 512 : (h + 1) * 512], hsT[h])
    hsT_sb = [hsT_all[:, d, :] for d in range(NDT)]
    wo_all = consts.tile([128, 2, D], BF16, tag="wo", name="wo_all")
    nc.sync.dma_start(wo_all[:], wo[:])
    wo_sb = [wo_all[:, p, :] for p in range(2)]

    # ---- PE warmup: dummy matmuls (no DMA deps) so HAM reaches 8/8 before
    # the real head matmuls issue; they run during the initial DMA wait ----
    warm = ps_s.tile([128, 128], F32, tag="smm", padded_shape=[128, 1024], name="warm")
    for i in range(28):
        nc.tensor.matmul(warm[:], lhsT=ones128[:], rhs=ones128[:], start=True, stop=True)

    qT_sb = [sb.tile([128, S], BF16, tag=f"qT{m}", name=f"qT{m}") for m in range(2)]
    kT_sb = [sb.tile([128, S], BF16, tag=f"kT{m}", name=f"kT{m}") for m in range(2)]
    v_sb = [sb.tile([128, GD], BF16, tag=f"v{st}", name=f"v{st}") for st in range(NKT)]
    # gating broadcast [128, 2*GQ] per 2-qc group; per-parity tiles
    gb_sb = [sb.tile([128, 2 * GQ], F32, tag=f"gb{h}", name=f"gb{h}") for h in range(2)]
    # scaled ctx^T bf16, alive into the following qc (O-projection)
    ctx_sc = [
        [sb.tile([128, GQ], BF16, tag=f"ctxs{p}_{par}", name=f"ctxs{p}_{par}") for p in range(2)]
        for par in range(2)
    ]

    def qk_proj(w_sb, dst, m, cols, bias_sb):
        """Project [128,1024] of Q^T or K^T (m selects the 128-row pair tile).
        A matmul output must fit one PSUM bank, so each 512-col half is its
        own accumulation group; the bias-add cast reads both banks at once."""
        p = ps_s.tile([128, 1024], F32, tag="smm", name="pp")
        for h in range(2):
            hs_ = slice(cols.start + h * 512, cols.start + (h + 1) * 512)
            for d in range(NDT):
                nc.tensor.matmul(
                    p[:, h * 512 : (h + 1) * 512],
                    lhsT=w_sb[d][:, m * 128 : (m + 1) * 128], rhs=hsT_sb[d][:, hs_],
                    start=(d == 0), stop=(d == NDT - 1),
                )
        nc.vector.tensor_scalar_add(dst[:, cols], p[:], bias_sb[:, m : m + 1])

    def v_proj(st, half):
        """Project the 128 V columns a single head-pair needs (half=pr), so
        each of the two qc0 pairs computes just its own half JIT."""
        vp = ps_s.tile([128, 128], F32, tag="smm", padded_shape=[128, 1024], name="vp")
        ss = slice(st * 128, (st + 1) * 128)
        hc = slice(half * 128, (half + 1) * 128)
        for d in range(NDT):
            nc.tensor.matmul(
                vp[:], lhsT=hsT_sb[d][:, ss], rhs=wv_sb[d][:, hc],
                start=(d == 0), stop=(d == NDT - 1),
            )
        nc.vector.tensor_tensor(v_sb[st][:, hc], vp[:], bv_bc[:, hc], op=OP.add)

    # ---- main loop: 8 pairs x 16 exp-paced slots, with interleaved extras ----
    # state carried between pairs for the deferred tail
    prev = None  # (qc, pr, ks2, ctxA, ctxB)

    def make_tail_extras(qc, pr, ks2, ctxA, ctxB, done=None):
        """Denominator matmuls + reciprocal + gating/denominator ctx scaling
        for pair (qc, pr), emitted as extras inside the following pair."""
        ex = []
        dn = {}

        def khalf():
            # collapse ks2's two slot-pairs with one wide DVE add; halves the
            # number of denominator matmul streams the PE must run
            dn["kh"] = wrk.tile([128, 2048], BF16, tag="kh", name="kh")
            nc.vector.tensor_tensor(dn["kh"][:], ks2[:, 0:2048], ks2[:, 2048:4096], op=OP.add)
        ex.append((1, 2, khalf))

        def dmm(j):
            def f():
                if j == 0:
                    dn["t"] = ps_s.tile([128, 1024], F32, tag="smm", name="dn")
                for h in range(2):
                    nc.tensor.matmul(
                        dn["t"][:, h * 512 : (h + 1) * 512], lhsT=ones128[:],
                        rhs=dn["kh"][:, j * 1024 + h * 512 : j * 1024 + (h + 1) * 512],
                        start=(j == 0), stop=(j == 1),
                    )
            return f

        for j in range(2):
            ex.append((2, 2, dmm(j)))

        def recip():
            dn["r"] = wrk.tile([128, 1024], F32, tag="rab", name="rab")
            nc.vector.reciprocal_approx_fast(dn["r"][:], dn["t"][:])
        ex.append((1, 2, recip))

        def scale1():
            dn["tmp"] = wrk.tile([128, GQ], F32, tag="tmp", name="tmp")
            nc.vector.tensor_tensor(
                dn["tmp"][0:64, :], ctxA[0:64, :], dn["r"][0:64, 0:GQ], op=OP.mult
            )

        def scale2():
            nc.vector.tensor_tensor(
                dn["tmp"][64:128, :], ctxB[64:128, :], dn["r"][64:128, GQ : 2 * GQ],
                op=OP.mult,
            )

        def scale3():
            gcols = slice((qc % 2) * GQ, (qc % 2) * GQ + GQ)
            nc.vector.tensor_tensor(
                ctx_sc[qc % 2][pr][:], dn["tmp"][:], gb_sb[(qc // 2) % 2][:, gcols],
                op=OP.mult,
            )
            if done is not None:
                done[0] = True
        ex += [(1, 2, scale1), (1, 2, scale2), (1, 2, scale3)]
        return ex

    def make_oproj_extras(qc, mts):
        """O-projection of chunk qc (reads ctx_sc[qc%2][*]) for column tiles
        `mts`, two extras each (matmul pair, then cast+store)."""
        ex = []
        cs = slice(qc * GQ, (qc + 1) * GQ)
        st = {}
        for mt in mts:
            def fmm(mt=mt):
                ms = slice(mt * 128, (mt + 1) * 128)
                o_ps = ps_s.tile(
                    [128, GQ], F32, tag="smm", padded_shape=[128, 1024], name="ops"
                )
                for p in range(2):
                    nc.tensor.matmul(
                        o_ps[:], lhsT=wo_sb[p][:, ms], rhs=ctx_sc[qc % 2][p][:],
                        start=(p == 0), stop=(p == 1),
                    )
                st["ps"] = o_ps

            def fout(mt=mt):
                # pair up column tiles into one [128, 1024] store (bigger
                # per-partition DMA runs -> fewer, fatter descriptors)
                if mt % 2 == 0:
                    st["ost"] = wrk.tile([128, 2 * GQ], F32, tag="ost", bufs=4, name="ost")
                nc.vector.tensor_copy(
                    st["ost"][:, (mt % 2) * GQ : (mt % 2 + 1) * GQ], st["ps"][:]
                )
                if mt % 2 == 1:
                    nc.sync.dma_start(outT[qc, mt // 2, :, :], st["ost"][:])
            ex += [(2, 0, fmm), (1, 0, fout)]
        return ex

    def make_proj_extras(w_sb, bias_sb, dst, m, half, split_cast=False):
        """Q^T/K^T projection m-tile for qc pair `half`, as extras (2 matmul
        half-groups + bias cast; split_cast casts each 512-col half as soon as
        its matmuls finish, so consumers do not wait on the later half's DMA)."""
        cols = slice(half * 1024, (half + 1) * 1024)
        st = {}

        def fh(h):
            def f():
                if h == 0:
                    st["p"] = ps_s.tile([128, 1024], F32, tag="smm", name="pp")
                hs_ = slice(cols.start + h * 512, cols.start + (h + 1) * 512)
                for d in range(NDT):
                    nc.tensor.matmul(
                        st["p"][:, h * 512 : (h + 1) * 512],
                        lhsT=w_sb[d][:, m * 128 : (m + 1) * 128],
                        rhs=hsT_sb[d][:, hs_],
                        start=(d == 0), stop=(d == NDT - 1),
                    )
            return f

        def fcast(h0, h1):
            def f():
                cc = slice(cols.start + h0 * 512, cols.start + h1 * 512)
                nc.vector.tensor_scalar_add(
                    dst[:, cc], st["p"][:, h0 * 512 : h1 * 512], bias_sb[:, m : m + 1]
                )
            return f

        if split_cast:
            return [(4, 0, fh(0)), (1, 0, fcast(0, 1)), (4, 0, fh(1)), (1, 0, fcast(1, 2))]
        return [(4, 0, fh(0)), (4, 0, fh(1)), (1, 0, fcast(0, 2))]

    def make_projq_extras(w_sb, bias_sb, dst, m, q):
        """Single 512-col projection group (quarter q) as [matmuls, cast]."""
        cc = slice(q * 512, (q + 1) * 512)
        st = {}

        def fmm():
            st["p"] = ps_s.tile([128, 512], F32, tag="smm", padded_shape=[128, 1024], name="pq")
            for d in range(NDT):
                nc.tensor.matmul(
                    st["p"][:], lhsT=w_sb[d][:, m * 128 : (m + 1) * 128],
                    rhs=hsT_sb[d][:, cc],
                    start=(d == 0), stop=(d == NDT - 1),
                )

        def fcast():
            nc.vector.tensor_scalar_add(dst[:, cc], st["p"][:], bias_sb[:, m : m + 1])
        return [(4, 0, fmm), (1, 0, fcast)]

    def make_gating_extras(g2):
        """Gating for qc pair g2, split into fine-grained extras."""
        cols = slice(g2 * 1024, (g2 + 1) * 1024)
        st = {}

        def fmean(h):
            def f():
                if h == 0:
                    st["gp"] = ps_s.tile(
                        [1, 1024], F32, tag="smm", padded_shape=[128, 1024], name="gp"
                    )
                hs_ = slice(cols.start + h * 512, cols.start + (h + 1) * 512)
                for d in range(NDT):
                    nc.tensor.matmul(
                        st["gp"][:, h * 512 : (h + 1) * 512], lhsT=ones128[:, 0:1],
                        rhs=hsT_sb[d][:, hs_],
                        start=(d == 0), stop=(d == NDT - 1),
                    )
            return f

        def fexp():
            st["e"] = wrk.tile([1, 1024], F32, tag="ge", name="ge")
            nc.scalar.activation(
                st["e"][:], st["gp"][:], AF.Exp, bias=gg_sb[:, 0:1], scale=gg_sb[:, 1:2]
            )

        def fsig():
            gt = wrk.tile([1, 1024], F32, tag="gt", name="gt")
            nc.vector.tensor_scalar_add(gt[:], st["e"][:], 1.0)
            st["g"] = wrk.tile([1, 1024], F32, tag="gr", name="gr")
            nc.vector.reciprocal_approx_fast(st["g"][:], gt[:])

        def fsplit():
            st["gh"] = wrk.tile([1, 1024], BF16, tag="gh", name="gh")
            nc.vector.tensor_copy(st["gh"][:], st["g"][:])
            gd = wrk.tile([1, 1024], F32, tag="gd", name="gd")
            nc.vector.tensor_tensor(gd[:], st["g"][:], st["gh"][:], op=OP.subtract)
            st["gl"] = wrk.tile([1, 1024], BF16, tag="gl", name="gl")
            nc.vector.tensor_copy(st["gl"][:], gd[:])

        def fbc(h):
            def f():
                if h == 0:
                    st["gbp"] = ps_s.tile([128, 1024], F32, tag="smm", name="gbp")
                hw = slice(h * 512, (h + 1) * 512)
                nc.tensor.matmul(
                    st["gbp"][:, hw], lhsT=ones1b[:], rhs=st["gh"][:, hw],
                    start=True, stop=False,
                )
                nc.tensor.matmul(
                    st["gbp"][:, hw], lhsT=ones1b[:], rhs=st["gl"][:, hw],
                    start=False, stop=True,
                )
            return f

        def fcopy():
            nc.vector.tensor_copy(gb_sb[g2 % 2][:], st["gbp"][:])
        return [(4, 0, fmean(0)), (4, 0, fmean(1)), (1, 0, fexp), (1, 0, fsig),
                (1, 0, fsplit), (1, 0, fbc(0)), (1, 0, fbc(1)), (1, 0, fcopy)]

    # ---- head phase: minimum work before exp #0 can issue ----
    # K^T and Q^T first halves (k tiles 0..7, q chunks 0..1); the K second
    # halves, gating and V run as extras / JIT inside the qc0 pairs.
    for m in range(2):
        qk_proj(wk_sb, kT_sb[m], m, slice(0, 1024), bk_sb)
    for m in range(2):
        qk_proj(wq_sb, qT_sb[m], m, slice(0, 1024), bq_sb)

    extras = []  # persistent work queue, popped a few items per slot
    # pair 0: gating(0) (its gb is read by the qc0 tails) and K m0's second
    # half (kT_sb[m] is read by pr=m only, so m1 can wait for pair 1)
    extras += make_gating_extras(0)
    extras += make_proj_extras(wk_sb, bk_sb, kT_sb[0], 0, 1, split_cast=True)
    for qc in range(NQC):
        cs = slice(qc * GQ, (qc + 1) * GQ)
        for pr in range(2):
            tail_done = [prev is None]
            if prev is not None:
                extras += make_tail_extras(*prev, done=tail_done)
            if qc == 0 and pr == 1:
                # pr1's K m1 second half must land before its k tile 8
                extras += make_proj_extras(wk_sb, bk_sb, kT_sb[1], 1, 1, split_cast=True)
            if pr == 0 and qc > 0:
                extras += make_oproj_extras(qc - 1, range(0, 4))
            if pr == 1 and qc > 0:
                extras += make_oproj_extras(qc - 1, range(4, 8))
            if qc == 1 and pr == 0:
                extras += make_proj_extras(wq_sb, bq_sb, qT_sb[0], 0, 1)
            if qc == 1 and pr == 1:
                extras += make_proj_extras(wq_sb, bq_sb, qT_sb[1], 1, 1)
            if qc == 2 and pr == 0:
                extras += make_gating_extras(1)
            jit_v = qc == 0

            ctxA = ps_c.tile([128, GQ], F32, tag="ctx", name="ctxA")
            ctxB = ps_c.tile([128, GQ], F32, tag="ctx", name="ctxB")
            ks2 = sb.tile([128, BKT * 1024], BF16, tag=f"ks2_{(qc * 2 + pr) % 2}", name="ks2")
            bigs = [None] * NBIG
            pend_av = []  # AV matmuls deferred while the prev tail drains ctx banks
            first_pair = prev is None

            def av(kt):
                big = bigs[kt // BKT]
                base = (kt % BKT) * 1024
                nc.tensor.matmul(
                    ctxA[0:64, :], lhsT=v_sb[kt][:, pr * 128 : pr * 128 + 64],
                    rhs=big[:, base : base + GQ], tile_position=(0, 0),
                    start=(kt == 0), stop=(kt == NKT - 1),
                )
                nc.tensor.matmul(
                    ctxB[64:128, :], lhsT=v_sb[kt][:, pr * 128 + 64 : pr * 128 + 128],
                    rhs=big[:, base + GQ : base + 2 * GQ], tile_position=(0, 64),
                    start=(kt == 0), stop=(kt == NKT - 1),
                )

            for kt in range(NKT):
                # scores pair -> 2-bank PSUM
                ks_ = slice(kt * 128, (kt + 1) * 128)
                sp = ps_s.tile([128, 2 * GQ], F32, tag="smm", name="smm")
                nc.tensor.matmul(
                    sp[:, 0:GQ], lhsT=kT_sb[pr][0:64, ks_], rhs=qT_sb[pr][0:64, cs],
                    tile_position=(0, 0), start=True, stop=True,
                )
                nc.tensor.matmul(
                    sp[:, GQ : 2 * GQ], lhsT=kT_sb[pr][64:128, ks_], rhs=qT_sb[pr][64:128, cs],
                    tile_position=(64, 0), start=True, stop=True,
                )
                # wide exp into the et big-tile slot
                if kt % BKT == 0:
                    bigs[kt // BKT] = etp.tile(
                        [128, BKT * 1024], BF16, tag="big", name="big"
                    )
                eslot = bigs[kt // BKT][:, (kt % BKT) * 1024 : (kt % BKT + 1) * 1024]
                nc.scalar.activation(eslot, sp[:], AF.Exp, bias=zbias[:, 0:1], scale=0.125)
                # extras fill the PE/DVE slack inside the exp-paced slot.
                # min_slot keeps the previous pair's denominator matmuls
                # (which depend on its last fold) from being emitted ahead of
                # scores(1..2), which would stall the in-order PE queue.
                budget = 3 if jit_v else 4
                while extras and budget > 0 and extras[0][1] <= kt:
                    w, _, f = extras.pop(0)
                    budget -= w
                    f()
                if jit_v:
                    v_proj(kt, pr)
                # AV of the previous kt (deferred while prev ctx banks drain)
                if kt > 0:
                    pend_av.append(kt - 1)
                    if tail_done[0]:
                        while pend_av:
                            av(pend_av.pop(0))
                # fold completed big tiles into ks2, half-width at a time; the
                # last big folds its first half early (after exp 13) so only
                # one [128,2048] add depends on the final exp of the pair.
                HW = BKT * 1024 // 2
                if kt == 2 * BKT - 1:
                    for h in range(2):
                        nc.vector.tensor_tensor(
                            ks2[:, h * HW : (h + 1) * HW], bigs[0][:, h * HW : (h + 1) * HW],
                            bigs[1][:, h * HW : (h + 1) * HW], op=OP.add,
                        )
                elif kt == 3 * BKT - 1:
                    for h in range(2):
                        nc.vector.tensor_tensor(
                            ks2[:, h * HW : (h + 1) * HW], ks2[:, h * HW : (h + 1) * HW],
                            bigs[2][:, h * HW : (h + 1) * HW], op=OP.add,
                        )
                elif kt == NKT - 3:
                    nc.vector.tensor_tensor(
                        ks2[:, 0:HW], ks2[:, 0:HW], bigs[NBIG - 1][:, 0:HW], op=OP.add
                    )
                elif kt == NKT - 1:
                    nc.vector.tensor_tensor(
                        ks2[:, HW:], ks2[:, HW:], bigs[NBIG - 1][:, HW:], op=OP.add
                    )
            while pend_av:
                av(pend_av.pop(0))
            av(NKT - 1)
            prev = (qc, pr, ks2, ctxA, ctxB)

    # ---- drain: leftover extras, last pair tail, last O-projection ----
    for _, _, f in extras + make_tail_extras(*prev) + make_oproj_extras(NQC - 1, range(8)):
        f()


def build_gau_nc(debug: bool = False):
    nc = bacc.Bacc("TRN2", target_bir_lowering=False, debug=debug, num_devices=NCORES)
    io = {
        "hsT": nc.dram_tensor("hsT", [4, 128, NDT, S // 4], BF16, kind="ExternalInput").ap(),
        "wq": nc.dram_tensor("wq", [128, NDT, GD], BF16, kind="ExternalInput").ap(),
        "wk": nc.dram_tensor("wk", [128, NDT, GD], BF16, kind="ExternalInput").ap(),
        "wv": nc.dram_tensor("wv", [128, NDT, GD], BF16, kind="ExternalInput").ap(),
        "wo": nc.dram_tensor("wo", [128, 2, D], BF16, kind="ExternalInput").ap(),
        "bq": nc.dram_tensor("bq", [128, 2], F32, kind="ExternalInput").ap(),
        "bk": nc.dram_tensor("bk", [128, 2], F32, kind="ExternalInput").ap(),
        "bv": nc.dram_tensor("bv", [128, GD], F32, kind="ExternalInput").ap(),
        "gg": nc.dram_tensor("gg", [2], F32, kind="ExternalInput").ap(),
        "outT": nc.dram_tensor("outT", [NQC, D // 256, 128, 2 * GQ], F32, kind="ExternalOutput").ap(),
    }
    with tile.TileContext(nc) as tc:
        with ExitStack() as ctx:
            _build(ctx, tc, io)
    nc.compile()
    return nc


def make_in_maps(hidden_states, Wq, bq, Wk, bk, Wv, bv, Wo, gating_factor, gating_bias):
    """Shard full inputs into 8 per-core input maps (host-side prep)."""
    bf = ml_dtypes.bfloat16
    f32 = np.float32
    hs = np.asarray(hidden_states, f32)
    Wq, Wk, Wv, Wo = (np.asarray(a, f32) for a in (Wq, Wk, Wv, Wo))
    bq, bk, bv = (np.asarray(a, f32) for a in (bq, bk, bv))
    gf = np.float32(np.asarray(gating_factor, f32)[0])
    gb = np.float32(np.asarray(gating_bias, f32)[0])

    # hsT as [2(col half), 128(part), NDT, 1024]: one contiguous block per
    # DMA with 2KB-per-partition runs
    hsT_b = [
        np.ascontiguousarray(
            hs[b].T.reshape(NDT, 128, 4, S // 4).transpose(2, 1, 0, 3)
        ).astype(bf)
        for b in range(B)
    ]

    def wtile(W):  # [D, cols] -> [128, NDT, cols]
        return np.ascontiguousarray(
            W.reshape(NDT, 128, W.shape[1]).transpose(1, 0, 2)
        ).astype(bf)

    in_maps = []
    for c in range(NCORES):
        b, g = divmod(c, NCORES // B)
        cols = slice(g * GD, (g + 1) * GD)
        in_maps.append(
            {
                "hsT": hsT_b[b],
                "wq": wtile(Wq[:, cols]),
                "wk": wtile(Wk[:, cols]),
                "wv": wtile(Wv[:, cols]),
                "wo": np.ascontiguousarray(
                    Wo[cols, :].reshape(2, 128, D).transpose(1, 0, 2)
                ).astype(bf),
                "bq": np.ascontiguousarray(bq[cols].reshape(2, 128).T),
                "bk": np.ascontiguousarray(bk[cols].reshape(2, 128).T),
                "bv": np.ascontiguousarray(np.broadcast_to(bv[cols], (128, GD))),
                "gg": np.array([-gb, -gf / D], f32),
            }
        )
    return in_maps


_NC_CACHE: dict = {}


def _get_nc():
    if "nc" not in _NC_CACHE:
        _NC_CACHE["nc"] = build_gau_nc()
    return _NC_CACHE["nc"]


def run_gau(in_maps, **kwargs):
    nc = _get_nc()
    return run_bass_kernel_spmd(nc, in_maps, core_ids=list(range(NCORES)), **kwargs)


def assemble_output(results, bo):
    """Sum per-batch head-group partials, transpose back, add bo."""
    bo = np.asarray(bo, np.float32)
    gpb = NCORES // B
    out = np.empty((B, S, D), np.float32)
    for b in range(B):
        acc = results[gpb * b]["outT"].astype(np.float32)
        for g in range(1, gpb):
            acc = acc + results[gpb * b + g]["outT"].astype(np.float32)
        # acc is [NQC, D//256, 128, 2*GQ] -> [D, S]
        acc = acc.reshape(NQC, D // 256, 128, 2, GQ).transpose(1, 3, 2, 0, 4).reshape(D, S)
        out[b] = acc.T + bo[None, :]
    return out


def kernel(hidden_states, Wq, bq, Wk, bk, Wv, bv, Wo, bo, gating_factor, gating_bias):
    in_maps = make_in_maps(
        hidden_states, Wq, bq, Wk, bk, Wv, bv, Wo, gating_factor, gating_bias
    )
    res = run_gau(in_maps)
    return assemble_output(res.results, bo)


# revision 33
# speedup vs baseline: 1.0519x; 1.0296x over previous
"""GAU attention (gated attention unit) Trainium2 Bass kernel.

Reference computation (B=2, S=2048, D=1024, H=16, DH=64):
    q = (hs @ Wq + bq), k = (hs @ Wk + bk), v = (hs @ Wv + bv)   per-head [B,S,H,DH]
    scores = q k^T / sqrt(DH);  probs = softmax(scores, axis=k)
    gating = sigmoid(gf * mean_d(hs) + gb)          # [B, S] per (batch, query)
    ctx = (probs * gating) @ v;  out = ctx @ Wo + bo

Sharding: 8 cores = 2 batches x 4 head-groups (4 heads each).  Each core
computes out^T partial [D, S] for its (batch, head-group); host sums the 4
partials per batch and adds bo.

v2 dataflow: the kernel is paced by the ACT engine (128 exp instructions of
[128,1024] at ~1us each, one per (qchunk, headpair, ktile)).  All other work
(Q/K/V/O projections, softmax denominators, gating, scaling) is emitted
interleaved into the exp-paced loop so PE/DVE fill the slack instead of
running in serial phases between attention blocks:
  - scores^T per (pair, kt): two row-packed K=64 matmuls -> 2-bank PSUM ->
    one wide exp -> et slot of a [128, 4*1024] bf16 "big" tile (4 kt each).
  - denominators: DVE folds the 4-kt big tiles (3 adds/pair instead of 15),
    then 4 accumulating ones128 matmuls broadcast the k-partition sum.
  - AV: col-packed matmuls, V stationary, accumulated over kt in PSUM; the
    first 4 AVs of a pair are emitted late so the previous pair's ctx scale
    (which shares the 2 ctx PSUM banks) never stalls the in-order PE queue.
  - per-pair tail (denom matmuls, reciprocal, gating+denominator scaling) and
    per-chunk O-projection run as "extras" popped inside the NEXT pair's loop.
  - gating sigmoid is computed as 1/(1+exp(-x)) so only the exp ACT table is
    ever loaded (no sigmoid table, no table thrash); host passes [-gb, -gf/D].
PSUM: scores 2banks x3 bufs + ctx 1bank x2 = 8 banks.  All transient matmul
outputs (K/Q/V-proj, denom, gating broadcast, O-proj) share the scores tag.
"""

import sys

for _p in ("/opt/trn_rl_repo", "/root/.axon_site/_ro/trn_rl_repo"):
    if _p not in sys.path:
        sys.path.append(_p)

from contextlib import ExitStack

import ml_dtypes
import numpy as np

import concourse.bass as bass
import concourse.mybir as mybir
import concourse.tile as tile
from concourse import bacc
from concourse.bass_utils import run_bass_kernel_spmd

BF16 = mybir.dt.bfloat16
F32 = mybir.dt.float32
AF = mybir.ActivationFunctionType
OP = mybir.AluOpType

B, S, D, H = 2, 2048, 1024, 16
DH = 64
HPC = 4  # heads per core
GD = HPC * DH  # 256 (head-group width)
NCORES = 8
NDT = D // 128  # 8 contraction tiles over D
GQ = 512  # q-chunk width
NQC = S // GQ  # 4 q chunks
NKT = S // 128  # 16 k tiles
BKT = 4  # k tiles per "big" et tile
NBIG = NKT // BKT  # 4


def _build(ctx: ExitStack, tc: "tile.TileContext", io: dict):
    nc = tc.nc
    hsT, wq, wk, wv, wo = io["hsT"], io["wq"], io["wk"], io["wv"], io["wo"]
    bq, bk, bv, gg, outT = io["bq"], io["bk"], io["bv"], io["gg"], io["outT"]

    consts = ctx.enter_context(tc.tile_pool(name="consts", bufs=1))
    sb = ctx.enter_context(tc.tile_pool(name="sb", bufs=1))
    etp = ctx.enter_context(tc.tile_pool(name="etp", bufs=3))
    wrk = ctx.enter_context(tc.tile_pool(name="wrk", bufs=2))
    # PSUM: smm [128,1024] = 2 banks x 3 bufs + ctx [128,512] = 1 bank x 2
    ps_s = ctx.enter_context(tc.tile_pool(name="ps_s", bufs=3, space="PSUM"))
    ps_c = ctx.enter_context(tc.tile_pool(name="ps_c", bufs=2, space="PSUM"))

    # ---- constants (memsets only; const DMAs go after the big loads —
    # the sync HWDGE ring is FIFO, so small DMAs first would delay them) ----
    ones128 = consts.tile([128, 128], BF16, tag="ones128", name="ones128")
    nc.vector.memset(ones128[:], 1.0)
    ones1b = consts.tile([1, 128], BF16, tag="ones1b", name="ones1b")
    nc.vector.memset(ones1b[:], 1.0)
    zbias = consts.tile([128, 1], F32, tag="zbias", name="zbias")
    nc.vector.memset(zbias[:], 0.0)

    # ---- load weights first (the head K/Q projections need them before the
    # bulk of hs^T), then hs^T in two column-halves.  All tensors arrive
    # host-tiled [128, d, cols] so each matrix is ONE large DMA with >=4KB
    # per-partition contiguous runs (DMA is descriptor-dominated below 1MB).
    wk_all = consts.tile([128, NDT, GD], BF16, tag="wk", name="wk_all")
    nc.sync.dma_start(wk_all[:], wk[:])
    wq_all = consts.tile([128, NDT, GD], BF16, tag="wq", name="wq_all")
    nc.sync.dma_start(wq_all[:], wq[:])
    wv_all = consts.tile([128, NDT, GD], BF16, tag="wv", name="wv_all")
    nc.sync.dma_start(wv_all[:], wv[:])
    wk_sb = [wk_all[:, d, :] for d in range(NDT)]
    wq_sb = [wq_all[:, d, :] for d in range(NDT)]
    wv_sb = [wv_all[:, d, :] for d in range(NDT)]
    # small consts used early in pair 0 go before hsT (all tiny)
    bv_bc = consts.tile([128, GD], F32, tag="bvbc", name="bvbc")
    nc.sync.dma_start(bv_bc[:], bv[:, :])
    gg_sb = consts.tile([1, 2], F32, tag="gg", name="gg")
    nc.sync.dma_start(gg_sb[:], gg[None, :])
    bq_sb = consts.tile([128, 2], F32, tag="bq", name="bq")
    nc.sync.dma_start(bq_sb[:], bq[:])
    bk_sb = consts.tile([128, 2], F32, tag="bk", name="bk")
    nc.sync.dma_start(bk_sb[:], bk[:])
    # hsT in 4 column-quarter DMAs (1MB each) so the head projections can
    # start on quarter 0 while the rest streams in
    hsT_all = sb.tile([128, NDT, S], BF16, tag="hsT", name="hsT_all")
    for h in range(4):
        nc.sync.dma_start(hsT_all[:, :, h * 512 : (h + 1) * 512], hsT[h])
    hsT_sb = [hsT_all[:, d, :] for d in range(NDT)]
    wo_all = consts.tile([128, 2, D], BF16, tag="wo", name="wo_all")
    nc.sync.dma_start(wo_all[:], wo[:])
    wo_sb = [wo_all[:, p, :] for p in range(2)]

    # ---- PE warmup: dummy matmuls (no DMA deps) so HAM reaches 8/8 before
    # the real head matmuls issue; they run during the initial DMA wait ----
    warm = ps_s.tile([128, 128], F32, tag="smm", padded_shape=[128, 1024], name="warm")
    for i in range(28):
        nc.tensor.matmul(warm[:], lhsT=ones128[:], rhs=ones128[:], start=True, stop=True)

    qT_sb = [sb.tile([128, S], BF16, tag=f"qT{m}", name=f"qT{m}") for m in range(2)]
    kT_sb = [sb.tile([128, S], BF16, tag=f"kT{m}", name=f"kT{m}") for m in range(2)]
    v_sb = [sb.tile([128, GD], BF16, tag=f"v{st}", name=f"v{st}") for st in range(NKT)]
    # gating broadcast [128, 2*GQ] per 2-qc group; per-parity tiles
    gb_sb = [sb.tile([128, 2 * GQ], F32, tag=f"gb{h}", name=f"gb{h}") for h in range(2)]
    # scaled ctx^T bf16, alive into the following qc (O-projection)
    ctx_sc = [
        [sb.tile([128, GQ], BF16, tag=f"ctxs{p}_{par}", name=f"ctxs{p}_{par}") for p in range(2)]
        for par in range(2)
    ]

    def qk_proj(w_sb, dst, m, cols, bias_sb):
        """Project [128,1024] of Q^T or K^T (m selects the 128-row pair tile).
        A matmul output must fit one PSUM bank, so each 512-col half is its
        own accumulation group; the bias-add cast reads both banks at once."""
        p = ps_s.tile([128, 1024], F32, tag="smm", name="pp")
        for h in range(2):
            hs_ = slice(cols.start + h * 512, cols.start + (h + 1) * 512)
            for d in range(NDT):
                nc.tensor.matmul(
                    p[:, h * 512 : (h + 1) * 512],
                    lhsT=w_sb[d][:, m * 128 : (m + 1) * 128], rhs=hsT_sb[d][:, hs_],
                    start=(d == 0), stop=(d == NDT - 1),
                )
        nc.vector.tensor_scalar_add(dst[:, cols], p[:], bias_sb[:, m : m + 1])

    def v_proj(st, half):
        """Project the 128 V columns a single head-pair needs (half=pr), so
        each of the two qc0 pairs computes just its own half JIT."""
        vp = ps_s.tile([128, 128], F32, tag="smm", padded_shape=[128, 1024], name="vp")
        ss = slice(st * 128, (st + 1) * 128)
        hc = slice(half * 128, (half + 1) * 128)
        for d in range(NDT):
            nc.tensor.matmul(
                vp[:], lhsT=hsT_sb[d][:, ss], rhs=wv_sb[d][:, hc],
                start=(d == 0), stop=(d == NDT - 1),
            )
        nc.vector.tensor_tensor(v_sb[st][:, hc], vp[:], bv_bc[:, hc], op=OP.add)

    # ---- main loop: 8 pairs x 16 exp-paced slots, with interleaved extras ----
    # state carried between pairs for the deferred tail
    prev = None  # (qc, pr, ks2, ctxA, ctxB)

    def make_tail_extras(qc, pr, ks2, ctxA, ctxB, done=None):
        """Denominator matmuls + reciprocal + gating/denominator ctx scaling
        for pair (qc, pr), emitted as extras inside the following pair."""
        ex = []
        dn = {}

        def dmm(j):
            def f():
                if j == 0:
                    dn["t"] = ps_s.tile([128, 1024], F32, tag="smm", name="dn")
                for h in range(2):
                    nc.tensor.matmul(
                        dn["t"][:, h * 512 : (h + 1) * 512], lhsT=ones128[:],
                        rhs=ks2[:, j * 1024 + h * 512 : j * 1024 + (h + 1) * 512],
                        start=(j == 0), stop=(j == NBIG - 1),
                    )
            return f

        for j in range(NBIG):
            ex.append((2, 2, dmm(j)))

        def recip():
            dn["r"] = wrk.tile([128, 1024], F32, tag="rab", name="rab")
            nc.vector.reciprocal_approx_fast(dn["r"][:], dn["t"][:])
        ex.append((1, 2, recip))

        def scale1():
            dn["tmp"] = wrk.tile([128, GQ], F32, tag="tmp", name="tmp")
            nc.vector.tensor_tensor(
                dn["tmp"][0:64, :], ctxA[0:64, :], dn["r"][0:64, 0:GQ], op=OP.mult
            )

        def scale2():
            nc.vector.tensor_tensor(
                dn["tmp"][64:128, :], ctxB[64:128, :], dn["r"][64:128, GQ : 2 * GQ],
                op=OP.mult,
            )

        def scale3():
            gcols = slice((qc % 2) * GQ, (qc % 2) * GQ + GQ)
            nc.vector.tensor_tensor(
                ctx_sc[qc % 2][pr][:], dn["tmp"][:], gb_sb[(qc // 2) % 2][:, gcols],
                op=OP.mult,
            )
            if done is not None:
                done[0] = True
        ex += [(1, 2, scale1), (1, 2, scale2), (1, 2, scale3)]
        return ex

    def make_oproj_extras(qc, mts):
        """O-projection of chunk qc (reads ctx_sc[qc%2][*]) for column tiles
        `mts`, two extras each (matmul pair, then cast+store)."""
        ex = []
        cs = slice(qc * GQ, (qc + 1) * GQ)
        st = {}
        for mt in mts:
            def fmm(mt=mt):
                ms = slice(mt * 128, (mt + 1) * 128)
                o_ps = ps_s.tile(
                    [128, GQ], F32, tag="smm", padded_shape=[128, 1024], name="ops"
                )
                for p in range(2):
                    nc.tensor.matmul(
                        o_ps[:], lhsT=wo_sb[p][:, ms], rhs=ctx_sc[qc % 2][p][:],
                        start=(p == 0), stop=(p == 1),
                    )
                st["ps"] = o_ps

            def fout(mt=mt):
                # pair up column tiles into one [128, 1024] store (bigger
                # per-partition DMA runs -> fewer, fatter descriptors)
                if mt % 2 == 0:
                    st["ost"] = wrk.tile([128, 2 * GQ], F32, tag="ost", bufs=4, name="ost")
                nc.vector.tensor_copy(
                    st["ost"][:, (mt % 2) * GQ : (mt % 2 + 1) * GQ], st["ps"][:]
                )
                if mt % 2 == 1:
                    nc.sync.dma_start(outT[qc, mt // 2, :, :], st["ost"][:])
            ex += [(2, 0, fmm), (1, 0, fout)]
        return ex

    def make_proj_extras(w_sb, bias_sb, dst, m, half, split_cast=False):
        """Q^T/K^T projection m-tile for qc pair `half`, as extras (2 matmul
        half-groups + bias cast; split_cast casts each 512-col half as soon as
        its matmuls finish, so consumers do not wait on the later half's DMA)."""
        cols = slice(half * 1024, (half + 1) * 1024)
        st = {}

        def fh(h):
            def f():
                if h == 0:
                    st["p"] = ps_s.tile([128, 1024], F32, tag="smm", name="pp")
                hs_ = slice(cols.start + h * 512, cols.start + (h + 1) * 512)
                for d in range(NDT):
                    nc.tensor.matmul(
                        st["p"][:, h * 512 : (h + 1) * 512],
                        lhsT=w_sb[d][:, m * 128 : (m + 1) * 128],
                        rhs=hsT_sb[d][:, hs_],
                        start=(d == 0), stop=(d == NDT - 1),
                    )
            return f

        def fcast(h0, h1):
            def f():
                cc = slice(cols.start + h0 * 512, cols.start + h1 * 512)
                nc.vector.tensor_scalar_add(
                    dst[:, cc], st["p"][:, h0 * 512 : h1 * 512], bias_sb[:, m : m + 1]
                )
            return f

        if split_cast:
            return [(4, 0, fh(0)), (1, 0, fcast(0, 1)), (4, 0, fh(1)), (1, 0, fcast(1, 2))]
        return [(4, 0, fh(0)), (4, 0, fh(1)), (1, 0, fcast(0, 2))]

    def make_projq_extras(w_sb, bias_sb, dst, m, q):
        """Single 512-col projection group (quarter q) as [matmuls, cast]."""
        cc = slice(q * 512, (q + 1) * 512)
        st = {}

        def fmm():
            st["p"] = ps_s.tile([128, 512], F32, tag="smm", padded_shape=[128, 1024], name="pq")
            for d in range(NDT):
                nc.tensor.matmul(
                    st["p"][:], lhsT=w_sb[d][:, m * 128 : (m + 1) * 128],
                    rhs=hsT_sb[d][:, cc],
                    start=(d == 0), stop=(d == NDT - 1),
                )

        def fcast():
            nc.vector.tensor_scalar_add(dst[:, cc], st["p"][:], bias_sb[:, m : m + 1])
        return [(4, 0, fmm), (1, 0, fcast)]

    def make_gating_extras(g2):
        """Gating for qc pair g2, split into fine-grained extras."""
        cols = slice(g2 * 1024, (g2 + 1) * 1024)
        st = {}

        def fmean(h):
            def f():
                if h == 0:
                    st["gp"] = ps_s.tile(
                        [1, 1024], F32, tag="smm", padded_shape=[128, 1024], name="gp"
                    )
                hs_ = slice(cols.start + h * 512, cols.start + (h + 1) * 512)
                for d in range(NDT):
                    nc.tensor.matmul(
                        st["gp"][:, h * 512 : (h + 1) * 512], lhsT=ones128[:, 0:1],
                        rhs=hsT_sb[d][:, hs_],
                        start=(d == 0), stop=(d == NDT - 1),
                    )
            return f

        def fexp():
            st["e"] = wrk.tile([1, 1024], F32, tag="ge", name="ge")
            nc.scalar.activation(
                st["e"][:], st["gp"][:], AF.Exp, bias=gg_sb[:, 0:1], scale=gg_sb[:, 1:2]
            )

        def fsig():
            gt = wrk.tile([1, 1024], F32, tag="gt", name="gt")
            nc.vector.tensor_scalar_add(gt[:], st["e"][:], 1.0)
            st["g"] = wrk.tile([1, 1024], F32, tag="gr", name="gr")
            nc.vector.reciprocal_approx_fast(st["g"][:], gt[:])

        def fsplit():
            st["gh"] = wrk.tile([1, 1024], BF16, tag="gh", name="gh")
            nc.vector.tensor_copy(st["gh"][:], st["g"][:])
            gd = wrk.tile([1, 1024], F32, tag="gd", name="gd")
            nc.vector.tensor_tensor(gd[:], st["g"][:], st["gh"][:], op=OP.subtract)
            st["gl"] = wrk.tile([1, 1024], BF16, tag="gl", name="gl")
            nc.vector.tensor_copy(st["gl"][:], gd[:])

        def fbc(h):
            def f():
                if h == 0:
                    st["gbp"] = ps_s.tile([128, 1024], F32, tag="smm", name="gbp")
                hw = slice(h * 512, (h + 1) * 512)
                nc.tensor.matmul(
                    st["gbp"][:, hw], lhsT=ones1b[:], rhs=st["gh"][:, hw],
                    start=True, stop=False,
                )
                nc.tensor.matmul(
                    st["gbp"][:, hw], lhsT=ones1b[:], rhs=st["gl"][:, hw],
                    start=False, stop=True,
                )
            return f

        def fcopy():
            nc.vector.tensor_copy(gb_sb[g2 % 2][:], st["gbp"][:])
        return [(4, 0, fmean(0)), (4, 0, fmean(1)), (1, 0, fexp), (1, 0, fsig),
                (1, 0, fsplit), (1, 0, fbc(0)), (1, 0, fbc(1)), (1, 0, fcopy)]

    # ---- head phase: minimum work before exp #0 can issue ----
    # K^T and Q^T first halves (k tiles 0..7, q chunks 0..1); the K second
    # halves, gating and V run as extras / JIT inside the qc0 pairs.
    for m in range(2):
        qk_proj(wk_sb, kT_sb[m], m, slice(0, 1024), bk_sb)
    for m in range(2):
        qk_proj(wq_sb, qT_sb[m], m, slice(0, 1024), bq_sb)

    extras = []  # persistent work queue, popped a few items per slot
    # pair 0: gating(0) (its gb is read by the qc0 tails) and K m0's second
    # half (kT_sb[m] is read by pr=m only, so m1 can wait for pair 1)
    extras += make_gating_extras(0)
    extras += make_proj_extras(wk_sb, bk_sb, kT_sb[0], 0, 1, split_cast=True)
    for qc in range(NQC):
        cs = slice(qc * GQ, (qc + 1) * GQ)
        for pr in range(2):
            tail_done = [prev is None]
            if prev is not None:
                extras += make_tail_extras(*prev, done=tail_done)
            if qc == 0 and pr == 1:
                # pr1's K m1 second half must land before its k tile 8
                extras += make_proj_extras(wk_sb, bk_sb, kT_sb[1], 1, 1, split_cast=True)
            if pr == 0 and qc > 0:
                extras += make_oproj_extras(qc - 1, range(0, 4))
            if pr == 1 and qc > 0:
                extras += make_oproj_extras(qc - 1, range(4, 8))
            if qc == 1 and pr == 0:
                extras += make_proj_extras(wq_sb, bq_sb, qT_sb[0], 0, 1)
            if qc == 1 and pr == 1:
                extras += make_proj_extras(wq_sb, bq_sb, qT_sb[1], 1, 1)
            if qc == 2 and pr == 0:
                extras += make_gating_extras(1)
            jit_v = qc == 0

            ctxA = ps_c.tile([128, GQ], F32, tag="ctx", name="ctxA")
            ctxB = ps_c.tile([128, GQ], F32, tag="ctx", name="ctxB")
            ks2 = sb.tile([128, BKT * 1024], BF16, tag=f"ks2_{(qc * 2 + pr) % 2}", name="ks2")
            bigs = [None] * NBIG
            pend_av = []  # AV matmuls deferred while the prev tail drains ctx banks
            first_pair = prev is None

            def av(kt):
                big = bigs[kt // BKT]
                base = (kt % BKT) * 1024
                nc.tensor.matmul(
                    ctxA[0:64, :], lhsT=v_sb[kt][:, pr * 128 : pr * 128 + 64],
                    rhs=big[:, base : base + GQ], tile_position=(0, 0),
                    start=(kt == 0), stop=(kt == NKT - 1),
                )
                nc.tensor.matmul(
                    ctxB[64:128, :], lhsT=v_sb[kt][:, pr * 128 + 64 : pr * 128 + 128],
                    rhs=big[:, base + GQ : base + 2 * GQ], tile_position=(0, 64),
                    start=(kt == 0), stop=(kt == NKT - 1),
                )

            for kt in range(NKT):
                # scores pair -> 2-bank PSUM
                ks_ = slice(kt * 128, (kt + 1) * 128)
                sp = ps_s.tile([128, 2 * GQ], F32, tag="smm", name="smm")
                nc.tensor.matmul(
                    sp[:, 0:GQ], lhsT=kT_sb[pr][0:64, ks_], rhs=qT_sb[pr][0:64, cs],
                    tile_position=(0, 0), start=True, stop=True,
                )
                nc.tensor.matmul(
                    sp[:, GQ : 2 * GQ], lhsT=kT_sb[pr][64:128, ks_], rhs=qT_sb[pr][64:128, cs],
                    tile_position=(64, 0), start=True, stop=True,
                )
                # wide exp into the et big-tile slot
                if kt % BKT == 0:
                    bigs[kt // BKT] = etp.tile(
                        [128, BKT * 1024], BF16, tag="big", name="big"
                    )
                eslot = bigs[kt // BKT][:, (kt % BKT) * 1024 : (kt % BKT + 1) * 1024]
                nc.scalar.activation(eslot, sp[:], AF.Exp, bias=zbias[:, 0:1], scale=0.125)
                # extras fill the PE/DVE slack inside the exp-paced slot.
                # min_slot keeps the previous pair's denominator matmuls
                # (which depend on its last fold) from being emitted ahead of
                # scores(1..2), which would stall the in-order PE queue.
                budget = (3 if jit_v else 4) if kt < NKT - 2 else 0
                while extras and budget > 0 and extras[0][1] <= kt:
                    w, _, f = extras.pop(0)
                    budget -= w
                    f()
                if jit_v:
                    v_proj(kt, pr)
                # AV of the previous kt (deferred while prev ctx banks drain)
                if kt > 0:
                    pend_av.append(kt - 1)
                    if tail_done[0]:
                        while pend_av:
                            av(pend_av.pop(0))
                # fold completed big tiles into ks2, half-width at a time; the
                # last big folds its first half early (after exp 13) so only
                # one [128,2048] add depends on the final exp of the pair.
                HW = BKT * 1024 // 2
                if kt == 2 * BKT - 1:
                    for h in range(2):
                        nc.vector.tensor_tensor(
                            ks2[:, h * HW : (h + 1) * HW], bigs[0][:, h * HW : (h + 1) * HW],
                            bigs[1][:, h * HW : (h + 1) * HW], op=OP.add,
                        )
                elif kt == 3 * BKT - 1:
                    for h in range(2):
                        nc.vector.tensor_tensor(
                            ks2[:, h * HW : (h + 1) * HW], ks2[:, h * HW : (h + 1) * HW],
                            bigs[2][:, h * HW : (h + 1) * HW], op=OP.add,
                        )
                elif kt == NKT - 3:
                    nc.vector.tensor_tensor(
                        ks2[:, 0:HW], ks2[:, 0:HW], bigs[NBIG - 1][:, 0:HW], op=OP.add
                    )
                elif kt >= NKT - 2:
                    # quarter-folds so the add depending on the very last exp
                    # of the pair is as small as possible (boundary latency)
                    q0 = HW + (kt - (NKT - 2)) * (HW // 2)
                    nc.vector.tensor_tensor(
                        ks2[:, q0 : q0 + HW // 2], ks2[:, q0 : q0 + HW // 2],
                        bigs[NBIG - 1][:, q0 : q0 + HW // 2], op=OP.add,
                    )
            while pend_av:
                av(pend_av.pop(0))
            av(NKT - 1)
            prev = (qc, pr, ks2, ctxA, ctxB)

    # ---- drain: leftover extras, last pair tail, last O-projection.
    # Dummy matmuls between items keep HAM at full clock through the
    # serial dependency chain (otherwise the drain runs at half rate). ----
    for _, _, f in extras + make_tail_extras(*prev) + make_oproj_extras(NQC - 1, range(8)):
        f()
        nc.tensor.matmul(warm[:], lhsT=ones128[:], rhs=ones128[:], start=True, stop=True)


def build_gau_nc(debug: bool = False):
    nc = bacc.Bacc("TRN2", target_bir_lowering=False, debug=debug, num_devices=NCORES)
    io = {
        "hsT": nc.dram_tensor("hsT", [4, 128, NDT, S // 4], BF16, kind="ExternalInput").ap(),
        "wq": nc.dram_tensor("wq", [128, NDT, GD], BF16, kind="ExternalInput").ap(),
        "wk": nc.dram_tensor("wk", [128, NDT, GD], BF16, kind="ExternalInput").ap(),
        "wv": nc.dram_tensor("wv", [128, NDT, GD], BF16, kind="ExternalInput").ap(),
        "wo": nc.dram_tensor("wo", [128, 2, D], BF16, kind="ExternalInput").ap(),
        "bq": nc.dram_tensor("bq", [128, 2], F32, kind="ExternalInput").ap(),
        "bk": nc.dram_tensor("bk", [128, 2], F32, kind="ExternalInput").ap(),
        "bv": nc.dram_tensor("bv", [128, GD], F32, kind="ExternalInput").ap(),
        "gg": nc.dram_tensor("gg", [2], F32, kind="ExternalInput").ap(),
        "outT": nc.dram_tensor("outT", [NQC, D // 256, 128, 2 * GQ], F32, kind="ExternalOutput").ap(),
    }
    with tile.TileContext(nc) as tc:
        with ExitStack() as ctx:
            _build(ctx, tc, io)
    nc.compile()
    return nc


def make_in_maps(hidden_states, Wq, bq, Wk, bk, Wv, bv, Wo, gating_factor, gating_bias):
    """Shard full inputs into 8 per-core input maps (host-side prep)."""
    bf = ml_dtypes.bfloat16
    f32 = np.float32
    hs = np.asarray(hidden_states, f32)
    Wq, Wk, Wv, Wo = (np.asarray(a, f32) for a in (Wq, Wk, Wv, Wo))
    bq, bk, bv = (np.asarray(a, f32) for a in (bq, bk, bv))
    gf = np.float32(np.asarray(gating_factor, f32)[0])
    gb = np.float32(np.asarray(gating_bias, f32)[0])

    # hsT as [2(col half), 128(part), NDT, 1024]: one contiguous block per
    # DMA with 2KB-per-partition runs
    hsT_b = [
        np.ascontiguousarray(
            hs[b].T.reshape(NDT, 128, 4, S // 4).transpose(2, 1, 0, 3)
        ).astype(bf)
        for b in range(B)
    ]

    def wtile(W):  # [D, cols] -> [128, NDT, cols]
        return np.ascontiguousarray(
            W.reshape(NDT, 128, W.shape[1]).transpose(1, 0, 2)
        ).astype(bf)

    in_maps = []
    for c in range(NCORES):
        b, g = divmod(c, NCORES // B)
        cols = slice(g * GD, (g + 1) * GD)
        in_maps.append(
            {
                "hsT": hsT_b[b],
                "wq": wtile(Wq[:, cols]),
                "wk": wtile(Wk[:, cols]),
                "wv": wtile(Wv[:, cols]),
                "wo": np.ascontiguousarray(
                    Wo[cols, :].reshape(2, 128, D).transpose(1, 0, 2)
                ).astype(bf),
                "bq": np.ascontiguousarray(bq[cols].reshape(2, 128).T),
                "bk": np.ascontiguousarray(bk[cols].reshape(2, 128).T),
                "bv": np.ascontiguousarray(np.broadcast_to(bv[cols], (128, GD))),
                "gg": np.array([-gb, -gf / D], f32),
            }
        )
    return in_maps


_NC_CACHE: dict = {}


def _get_nc():
    if "nc" not in _NC_CACHE:
        _NC_CACHE["nc"] = build_gau_nc()
    return _NC_CACHE["nc"]


def run_gau(in_maps, **kwargs):
    nc = _get_nc()
    return run_bass_kernel_spmd(nc, in_maps, core_ids=list(range(NCORES)), **kwargs)


def assemble_output(results, bo):
    """Sum per-batch head-group partials, transpose back, add bo."""
    bo = np.asarray(bo, np.float32)
    gpb = NCORES // B
    out = np.empty((B, S, D), np.float32)
    for b in range(B):
        acc = results[gpb * b]["outT"].astype(np.float32)
        for g in range(1, gpb):
            acc = acc + results[gpb * b + g]["outT"].astype(np.float32)
        # acc is [NQC, D//256, 128, 2*GQ] -> [D, S]
        acc = acc.reshape(NQC, D // 256, 128, 2, GQ).transpose(1, 3, 2, 0, 4).reshape(D, S)
        out[b] = acc.T + bo[None, :]
    return out


def kernel(hidden_states, Wq, bq, Wk, bk, Wv, bv, Wo, bo, gating_factor, gating_bias):
    in_maps = make_in_maps(
        hidden_states, Wq, bq, Wk, bk, Wv, bv, Wo, gating_factor, gating_bias
    )
    res = run_gau(in_maps)
    return assemble_output(res.results, bo)
